# revision 8
# baseline (speedup 1.0000x reference)
"""Trainium2 Bass kernel for the dense GNN message-passing step.

Computation (N=16384, NUM_IN=1024, NUM_OUT=256):
    states = zeros(N); states[input_indices] = input_values
    total  = states @ W + biases                      # GEMV over [N, N] f32
    out    = act_select(total)[output_indices]        # 0=id, 1=relu, 2=softsign

Strategy:
  * `states` is zero outside the (<=1024) positions named by input_indices,
    so only those rows of W contribute to the GEMV (host packs the live
    rows, padded to K=1024).
  * Only output_indices (256 of 16384) of the result are ever read, so
    only those COLUMNS of W are needed: the host gathers
    W[rows][:, output_indices] -> [1024, 256] (0.5 MB as fp16 instead of
    1 GB), and the device GEMV + bias + activation runs on just those
    neurons. The gathered column order IS the output order, so the
    host-side epilogue is a plain concat of the per-core slices.
  * The 256 columns are sharded 32-per-core across the 8 cores (tensor
    parallel per the sharding hint, applied post-gather).
  * Single-pass fp16 GEMV: x and W rounded to fp16, products accumulate
    exactly in f32 PSUM. Measured end-to-end error ~1.3e-4 relative
    (vs the 2e-2 gate); bias is added exactly via an f32 ones-row
    matmul into the same accumulation group.
  * Instruction count is kept minimal: the profile shows a fixed
    end-of-kernel drain that costs ~115 ns per queue instruction, so
    fewer instructions shrink the measured window directly.
  * Epilogue on [1,32] straight out of PSUM: |t| -> 1+|t| ->
    reciprocal_approx_fast (~51 ULP, plenty here) -> t*recip, with
    relu/softsign/identity selected by host-precomputed uint8 masks.
"""

import numpy as np
from contextlib import ExitStack

import concourse.bacc as bacc
import concourse.tile as tile
from concourse import mybir
from concourse.bass_utils import run_bass_kernel_spmd

N_CORES = 8
K = 1024                 # padded contraction size (live rows)
KC = K // 128            # 8 k-chunks
NOUT = 256               # gathered output neurons
NPC = NOUT // N_CORES    # 32 output columns per core
F32 = mybir.dt.float32
F16 = mybir.dt.float16
U8 = mybir.dt.uint8

_BUILT = None            # cached nc so repeat calls reuse the compiled module
LAST_RESULTS = None      # BassKernelResults of the most recent run (for test.py)


def _build_bass():
    nc = bacc.Bacc(
        "TRN2", target_bir_lowering=False, debug=False, num_devices=N_CORES
    )
    # w layout: [p, kc*col] — 512 B contiguous per partition.
    w = nc.dram_tensor("w", [128, KC * NPC], F16, kind="ExternalInput").ap()
    xh = nc.dram_tensor("xh", [128, KC], F16, kind="ExternalInput").ap()
    b = nc.dram_tensor("b", [1, NPC], F32, kind="ExternalInput").ap()
    m1 = nc.dram_tensor("m1", [1, NPC], U8, kind="ExternalInput").ap()
    m2 = nc.dram_tensor("m2", [1, NPC], U8, kind="ExternalInput").ap()
    o = nc.dram_tensor("o", [1, NPC], F32, kind="ExternalOutput").ap()

    with tile.TileContext(nc) as tc:
        with ExitStack() as ctx:
            small = ctx.enter_context(tc.tile_pool(name="small", bufs=1))
            wpool = ctx.enter_context(tc.tile_pool(name="wp", bufs=1))
            ppool = ctx.enter_context(tc.tile_pool(name="pp", bufs=1, space="PSUM"))
            scratch = ctx.enter_context(tc.tile_pool(name="scr", bufs=1))

            # Spread input DMAs across queues so their descriptor
            # generations (~0.7 us each of sequencer time) run in
            # parallel; W split in two keeps the DMA engines streaming
            # densely (single big DMAs were observed to trickle).
            xh_t = small.tile([128, KC], F16, tag="xh")
            nc.scalar.dma_start(xh_t[:], xh[:])
            w_t = wpool.tile([128, KC * NPC], F16, tag="w")
            q = 2 * NPC
            for i in range(4):
                nc.sync.dma_start(w_t[:, i * q : (i + 1) * q], w[:, i * q : (i + 1) * q])
            b_t = small.tile([1, NPC], F32, tag="bt")
            nc.gpsimd.dma_start(b_t[:], b[:])
            m1_t = small.tile([1, NPC], U8, tag="m1t")
            nc.gpsimd.dma_start(m1_t[:], m1[:])
            m2_t = small.tile([1, NPC], U8, tag="m2t")
            nc.gpsimd.dma_start(m2_t[:], m2[:])
            ones_t = small.tile([1, 1], F32, tag="ones")
            nc.gpsimd.memset(ones_t[:], 1.0)

            p = ppool.tile([1, NPC], F32, tag="p")

            # p = b + x'W  (bias exact via f32 ones-row matmul)
            nc.tensor.matmul(
                p[0:1, :], ones_t[0:1, :], b_t[0:1, :],
                start=True, stop=False,
            )
            for kc in range(KC):
                nc.tensor.matmul(
                    p[0:1, :], xh_t[:, kc : kc + 1],
                    w_t[:, kc * NPC : (kc + 1) * NPC],
                    start=False, stop=(kc == KC - 1),
                )

            # t = p; softsign chain on DVE, identity/relu copies on ACT.
            at = scratch.tile([1, NPC], F32, tag="at")
            nc.scalar.activation(                        # |t|      (ACT)
                at[:], p[0:1, :], mybir.ActivationFunctionType.Abs
            )
            ot = scratch.tile([1, NPC], F32, tag="ot")
            nc.scalar.copy(ot[:], p[0:1, :])             # t        (ACT)
            rt = scratch.tile([1, NPC], F32, tag="rt")
            nc.scalar.activation(                        # relu(t)  (ACT)
                rt[:], p[0:1, :], mybir.ActivationFunctionType.Relu
            )
            a1 = scratch.tile([1, NPC], F32, tag="a1")
            nc.vector.scalar_tensor_tensor(              # 1+|t|    (DVE)
                a1[:], at[:], 1.0, at[:],
                mybir.AluOpType.add, mybir.AluOpType.bypass,
            )
            vt = scratch.tile([1, NPC], F32, tag="vt")
            nc.vector.reciprocal_approx_fast(out=vt[:], in_=a1[:])
            sst = scratch.tile([1, NPC], F32, tag="sst")
            nc.vector.tensor_mul(sst[:], p[0:1, :], vt[:])  # softsign(t)
            nc.vector.copy_predicated(ot[:], m1_t[0:1, :], rt[:])
            nc.vector.copy_predicated(ot[:], m2_t[0:1, :], sst[:])

            nc.scalar.dma_start(o[:], ot[:])

    nc.compile()
    return nc


def kernel(**inputs) -> np.ndarray:
    global _BUILT, LAST_RESULTS

    iv = np.asarray(inputs["input_values"], dtype=np.float32)
    W = np.asarray(inputs["weight_matrix"], dtype=np.float32)
    bias = np.asarray(inputs["biases"], dtype=np.float32)
    act = np.asarray(inputs["act_ids"])
    iidx = np.asarray(inputs["input_indices"]).astype(np.int64)
    oidx = np.asarray(inputs["output_indices"]).astype(np.int64)

    n = W.shape[0]
    # Dense neuron-state vector (duplicate indices: last write wins, matching
    # jax's .at[].set) and its index support.
    states = np.zeros(n, np.float32)
    states[iidx] = iv
    live = np.zeros(n, dtype=bool)
    live[iidx] = True
    support = np.flatnonzero(live)
    assert support.size <= K, "more than K live rows not supported"
    rows = np.zeros(K, np.int64)          # pad with row 0 (x=0 there => no-op)
    rows[: support.size] = support
    xvec = np.zeros(K, np.float32)
    xvec[: support.size] = states[support]

    xh_t = np.ascontiguousarray(
        xvec.astype(np.float16).reshape(KC, 128).T
    )                                                     # [128, KC]

    in_maps = []
    for c in range(N_CORES):
        cols = oidx[c * NPC : (c + 1) * NPC]
        ws = W[np.ix_(rows, cols)].astype(np.float16)     # [K, NPC]
        # [K, NPC] -> [p, kc, col]: partition p holds row kc*128+p
        wh = np.ascontiguousarray(
            ws.reshape(KC, 128, NPC).transpose(1, 0, 2)
        ).reshape(128, KC * NPC)
        in_maps.append(
            {
                "w": wh,
                "xh": xh_t,
                "b": bias[cols].reshape(1, NPC).astype(np.float32),
                "m1": (act[cols] == 1).astype(np.uint8).reshape(1, NPC),
                "m2": (act[cols] == 2).astype(np.uint8).reshape(1, NPC),
            }
        )

    if _BUILT is None:
        _BUILT = _build_bass()
    LAST_RESULTS = run_bass_kernel_spmd(
        _BUILT, in_maps, core_ids=list(range(N_CORES))
    )
    return np.concatenate(
        [LAST_RESULTS.results[c]["o"][0] for c in range(N_CORES)]
    ).astype(np.float32)


# revision 10
# speedup vs baseline: 1.0078x; 1.0078x over previous
"""Trainium2 Bass kernel for the dense GNN message-passing step.

Computation (N=16384, NUM_IN=1024, NUM_OUT=256):
    states = zeros(N); states[input_indices] = input_values
    total  = states @ W + biases                      # GEMV over [N, N] f32
    out    = act_select(total)[output_indices]        # 0=id, 1=relu, 2=softsign

Strategy:
  * `states` is zero outside the (<=1024) positions named by input_indices,
    so only those rows of W contribute to the GEMV (host packs the live
    rows, padded to K=1024).
  * Only output_indices (256 of 16384) of the result are ever read, so
    only those COLUMNS of W are needed: the host gathers
    W[rows][:, output_indices] -> [1024, 256] (0.5 MB as fp16 instead of
    1 GB), and the device GEMV + bias + activation runs on just those
    neurons. The gathered column order IS the output order, so the
    host-side epilogue is a plain concat of the per-core slices.
  * The 256 columns are sharded 32-per-core across the 8 cores (tensor
    parallel per the sharding hint, applied post-gather).
  * Single-pass fp16 GEMV: x and W rounded to fp16, products accumulate
    exactly in f32 PSUM. Measured end-to-end error ~1.3e-4 relative
    (vs the 2e-2 gate); bias is added exactly via an f32 ones-row
    matmul into the same accumulation group.
  * Instruction count is kept minimal: the profile shows a fixed
    end-of-kernel drain that costs ~115 ns per queue instruction, so
    fewer instructions shrink the measured window directly.
  * Epilogue on [1,32] straight out of PSUM: |t| -> 1+|t| ->
    reciprocal_approx_fast (~51 ULP, plenty here) -> t*recip, with
    relu/softsign/identity selected by host-precomputed uint8 masks.
"""

import numpy as np
from contextlib import ExitStack

import concourse.bacc as bacc
import concourse.tile as tile
from concourse import mybir
from concourse.bass_utils import run_bass_kernel_spmd

N_CORES = 8
K = 1024                 # padded contraction size (live rows)
KC = K // 128            # 8 k-chunks
NOUT = 256               # gathered output neurons
NPC = NOUT // N_CORES    # 32 output columns per core
F32 = mybir.dt.float32
F16 = mybir.dt.float16
U8 = mybir.dt.uint8

_BUILT = None            # cached nc so repeat calls reuse the compiled module
LAST_RESULTS = None      # BassKernelResults of the most recent run (for test.py)


def _build_bass():
    nc = bacc.Bacc(
        "TRN2", target_bir_lowering=False, debug=False, num_devices=N_CORES
    )
    # w layout: [p, kc*col] — 512 B contiguous per partition.
    w = nc.dram_tensor("w", [128, KC * NPC], F16, kind="ExternalInput").ap()
    xh = nc.dram_tensor("xh", [128, KC], F16, kind="ExternalInput").ap()
    b = nc.dram_tensor("b", [1, NPC], F32, kind="ExternalInput").ap()
    m1 = nc.dram_tensor("m1", [1, NPC], U8, kind="ExternalInput").ap()
    m2 = nc.dram_tensor("m2", [1, NPC], U8, kind="ExternalInput").ap()
    o = nc.dram_tensor("o", [1, NPC], F32, kind="ExternalOutput").ap()

    with tile.TileContext(nc) as tc:
        with ExitStack() as ctx:
            small = ctx.enter_context(tc.tile_pool(name="small", bufs=1))
            wpool = ctx.enter_context(tc.tile_pool(name="wp", bufs=1))
            ppool = ctx.enter_context(tc.tile_pool(name="pp", bufs=1, space="PSUM"))
            scratch = ctx.enter_context(tc.tile_pool(name="scr", bufs=1))

            # Spread input DMAs across queues so their descriptor
            # generations (~0.7 us each of sequencer time) run in
            # parallel; W split in two keeps the DMA engines streaming
            # densely (single big DMAs were observed to trickle).
            xh_t = small.tile([128, KC], F16, tag="xh")
            nc.scalar.dma_start(xh_t[:], xh[:])
            w_t = wpool.tile([128, KC * NPC], F16, tag="w")
            half = (KC // 2) * NPC
            nc.sync.dma_start(w_t[:, 0:half], w[:, 0:half])
            nc.sync.dma_start(w_t[:, half:], w[:, half:])
            ones_t = small.tile([1, 1], F32, tag="ones")
            nc.gpsimd.memset(ones_t[:], 1.0)
            b_t = small.tile([1, NPC], F32, tag="bt")
            nc.gpsimd.dma_start(b_t[:], b[:])
            m1_t = small.tile([1, NPC], U8, tag="m1t")
            nc.gpsimd.dma_start(m1_t[:], m1[:])
            m2_t = small.tile([1, NPC], U8, tag="m2t")
            nc.gpsimd.dma_start(m2_t[:], m2[:])

            p = ppool.tile([1, NPC], F32, tag="p")

            # p = b + x'W  (bias exact via f32 ones-row matmul)
            nc.tensor.matmul(
                p[0:1, :], ones_t[0:1, :], b_t[0:1, :],
                start=True, stop=False,
            )
            for kc in range(KC):
                nc.tensor.matmul(
                    p[0:1, :], xh_t[:, kc : kc + 1],
                    w_t[:, kc * NPC : (kc + 1) * NPC],
                    start=False, stop=(kc == KC - 1),
                )

            # t = p; softsign chain on DVE, identity/relu copies on ACT.
            at = scratch.tile([1, NPC], F32, tag="at")
            nc.scalar.activation(                        # |t|      (ACT)
                at[:], p[0:1, :], mybir.ActivationFunctionType.Abs
            )
            ot = scratch.tile([1, NPC], F32, tag="ot")
            nc.scalar.copy(ot[:], p[0:1, :])             # t        (ACT)
            rt = scratch.tile([1, NPC], F32, tag="rt")
            nc.scalar.activation(                        # relu(t)  (ACT)
                rt[:], p[0:1, :], mybir.ActivationFunctionType.Relu
            )
            a1 = scratch.tile([1, NPC], F32, tag="a1")
            nc.vector.scalar_tensor_tensor(              # 1+|t|    (DVE)
                a1[:], at[:], 1.0, at[:],
                mybir.AluOpType.add, mybir.AluOpType.bypass,
            )
            vt = scratch.tile([1, NPC], F32, tag="vt")
            nc.vector.reciprocal_approx_fast(out=vt[:], in_=a1[:])
            sst = scratch.tile([1, NPC], F32, tag="sst")
            nc.vector.tensor_mul(sst[:], p[0:1, :], vt[:])  # softsign(t)
            nc.vector.copy_predicated(ot[:], m1_t[0:1, :], rt[:])
            nc.vector.copy_predicated(ot[:], m2_t[0:1, :], sst[:])

            nc.scalar.dma_start(o[:], ot[:])

    nc.compile()
    return nc


def kernel(**inputs) -> np.ndarray:
    global _BUILT, LAST_RESULTS

    iv = np.asarray(inputs["input_values"], dtype=np.float32)
    W = np.asarray(inputs["weight_matrix"], dtype=np.float32)
    bias = np.asarray(inputs["biases"], dtype=np.float32)
    act = np.asarray(inputs["act_ids"])
    iidx = np.asarray(inputs["input_indices"]).astype(np.int64)
    oidx = np.asarray(inputs["output_indices"]).astype(np.int64)

    n = W.shape[0]
    # Dense neuron-state vector (duplicate indices: last write wins, matching
    # jax's .at[].set) and its index support.
    states = np.zeros(n, np.float32)
    states[iidx] = iv
    live = np.zeros(n, dtype=bool)
    live[iidx] = True
    support = np.flatnonzero(live)
    assert support.size <= K, "more than K live rows not supported"
    rows = np.zeros(K, np.int64)          # pad with row 0 (x=0 there => no-op)
    rows[: support.size] = support
    xvec = np.zeros(K, np.float32)
    xvec[: support.size] = states[support]

    xh_t = np.ascontiguousarray(
        xvec.astype(np.float16).reshape(KC, 128).T
    )                                                     # [128, KC]

    in_maps = []
    for c in range(N_CORES):
        cols = oidx[c * NPC : (c + 1) * NPC]
        ws = W[np.ix_(rows, cols)].astype(np.float16)     # [K, NPC]
        # [K, NPC] -> [p, kc, col]: partition p holds row kc*128+p
        wh = np.ascontiguousarray(
            ws.reshape(KC, 128, NPC).transpose(1, 0, 2)
        ).reshape(128, KC * NPC)
        in_maps.append(
            {
                "w": wh,
                "xh": xh_t,
                "b": bias[cols].reshape(1, NPC).astype(np.float32),
                "m1": (act[cols] == 1).astype(np.uint8).reshape(1, NPC),
                "m2": (act[cols] == 2).astype(np.uint8).reshape(1, NPC),
            }
        )

    if _BUILT is None:
        _BUILT = _build_bass()
    LAST_RESULTS = run_bass_kernel_spmd(
        _BUILT, in_maps, core_ids=list(range(N_CORES))
    )
    return np.concatenate(
        [LAST_RESULTS.results[c]["o"][0] for c in range(N_CORES)]
    ).astype(np.float32)


# revision 42
# speedup vs baseline: 1.0229x; 1.0151x over previous
"""Trainium2 Bass kernel for the dense GNN message-passing step.

Computation (N=16384, NUM_IN=1024, NUM_OUT=256):
    states = zeros(N); states[input_indices] = input_values
    total  = states @ W + biases                      # GEMV over [N, N] f32
    out    = act_select(total)[output_indices]        # 0=id, 1=relu, 2=softsign

Strategy:
  * `states` is zero outside the (<=1024) positions named by input_indices,
    so only those rows of W contribute to the GEMV (host packs the live
    rows, padded to K=1024).
  * Only output_indices (256 of 16384) of the result are ever read, so
    only those COLUMNS of W are needed: the host gathers
    W[rows][:, output_indices] -> [1024, 256] (0.5 MB as fp16 instead of
    1 GB), and the device GEMV + bias + activation runs on just those
    neurons. The gathered column order IS the output order, so the
    host-side epilogue is a plain concat of the per-core slices.
  * The 256 columns are sharded 32-per-core across the 8 cores (tensor
    parallel per the sharding hint, applied post-gather).
  * Single-pass fp16 GEMV: x and W rounded to fp16, products accumulate
    exactly in f32 PSUM. Measured end-to-end error ~1.3e-4 relative
    (vs the 2e-2 gate); bias is added exactly via an f32 ones-row
    matmul into the same accumulation group.
  * Instruction count is kept minimal: the profile shows a fixed
    end-of-kernel drain that costs ~115 ns per queue instruction, so
    fewer instructions shrink the measured window directly.
  * Epilogue on [1,32] straight out of PSUM: |t| -> 1+|t| ->
    reciprocal_approx_fast (~51 ULP, plenty here) -> t*recip, with
    relu/softsign/identity selected by host-precomputed uint8 masks.
"""

import sys
import types

import numpy as np
from contextlib import ExitStack

import concourse.bacc as bacc
import concourse.tile as tile
from concourse import mybir
from concourse.bass_utils import run_bass_kernel_spmd


def _ensure_ntff_hook_module():
    """bass_utils imports antenv.axon_hooks when BASS_TRACE=1; some agent
    images ship antenv without that submodule, which would crash the run
    instead of degrading to trace-skip. Install a shim (backed by
    trn_agent_boot's ctypes NTFF driver when present) only if the real
    module is missing."""
    try:
        import antenv.axon_hooks  # noqa: F401
        return
    except ImportError:
        pass
    hook = [None]
    mod = types.ModuleType("antenv.axon_hooks")
    mod.set_axon_ntff_profile_hook = lambda h: hook.__setitem__(0, h)
    mod.get_axon_ntff_profile_hook = lambda: hook[0]
    try:
        import antenv as _antenv
        from trn_agent_boot.trn_boot import _ntff_profile_via_ctypes

        mod.set_axon_ntff_profile_hook(
            _ntff_profile_via_ctypes("/opt/axon/libaxon_pjrt.so")
        )
        sys.modules["antenv.axon_hooks"] = mod
        _antenv.axon_hooks = mod
    except Exception:
        sys.modules.setdefault("antenv.axon_hooks", mod)


_ensure_ntff_hook_module()

N_CORES = 8
K = 1024                 # padded contraction size (live rows)
KC = K // 128            # 8 k-chunks
NOUT = 256               # gathered output neurons
NPC = NOUT // N_CORES    # 32 output columns per core
F32 = mybir.dt.float32
F16 = mybir.dt.float16
U8 = mybir.dt.uint8

_BUILT = None            # cached nc so repeat calls reuse the compiled module
LAST_RESULTS = None      # BassKernelResults of the most recent run (for test.py)


def _build_bass():
    nc = bacc.Bacc(
        "TRN2", target_bir_lowering=False, debug=False, num_devices=N_CORES
    )
    # w layout: [p, kc*col] — 512 B contiguous per partition.
    w = nc.dram_tensor("w", [128, KC * NPC], F16, kind="ExternalInput").ap()
    xh = nc.dram_tensor("xh", [128, KC], F16, kind="ExternalInput").ap()
    b = nc.dram_tensor("b", [1, NPC], F32, kind="ExternalInput").ap()
    m1 = nc.dram_tensor("m1", [1, NPC], U8, kind="ExternalInput").ap()
    m2 = nc.dram_tensor("m2", [1, NPC], U8, kind="ExternalInput").ap()
    o = nc.dram_tensor("o", [1, NPC], F32, kind="ExternalOutput").ap()

    with tile.TileContext(nc) as tc:
        with ExitStack() as ctx:
            small = ctx.enter_context(tc.tile_pool(name="small", bufs=1))
            wpool = ctx.enter_context(tc.tile_pool(name="wp", bufs=1))
            ppool = ctx.enter_context(tc.tile_pool(name="pp", bufs=1, space="PSUM"))
            scratch = ctx.enter_context(tc.tile_pool(name="scr", bufs=1))

            # Spread input DMAs across queues so their descriptor
            # generations (~0.7 us each of sequencer time) run in
            # parallel; W split in two keeps the DMA engines streaming
            # densely (single big DMAs were observed to trickle).
            xh_t = small.tile([128, KC], F16, tag="xh")
            nc.scalar.dma_start(xh_t[:], xh[:])
            w_t = wpool.tile([128, KC * NPC], F16, tag="w")
            half = (KC // 2) * NPC
            nc.sync.dma_start(w_t[:, 0:half], w[:, 0:half])
            nc.sync.dma_start(w_t[:, half:], w[:, half:])
            b_t = small.tile([1, NPC], F32, tag="bt")
            nc.gpsimd.dma_start(b_t[:], b[:])
            m1_t = small.tile([1, NPC], U8, tag="m1t")
            nc.gpsimd.dma_start(m1_t[:], m1[:])
            m2_t = small.tile([1, NPC], U8, tag="m2t")
            nc.gpsimd.dma_start(m2_t[:], m2[:])
            ones_t = small.tile([1, 1], F32, tag="ones")
            nc.gpsimd.memset(ones_t[:], 1.0)

            p = ppool.tile([1, NPC], F32, tag="p")

            # p = b + x'W  (bias exact via f32 ones-row matmul)
            nc.tensor.matmul(
                p[0:1, :], ones_t[0:1, :], b_t[0:1, :],
                start=True, stop=False,
            )
            for kc in range(KC):
                nc.tensor.matmul(
                    p[0:1, :], xh_t[:, kc : kc + 1],
                    w_t[:, kc * NPC : (kc + 1) * NPC],
                    start=False, stop=(kc == KC - 1),
                )

            # t = p; softsign chain on DVE, identity/relu copies on ACT.
            at = scratch.tile([1, NPC], F32, tag="at")
            nc.scalar.activation(                        # |t|      (ACT)
                at[:], p[0:1, :], mybir.ActivationFunctionType.Abs
            )
            ot = scratch.tile([1, NPC], F32, tag="ot")
            nc.scalar.copy(ot[:], p[0:1, :])             # t        (ACT)
            rt = scratch.tile([1, NPC], F32, tag="rt")
            nc.scalar.activation(                        # relu(t)  (ACT)
                rt[:], p[0:1, :], mybir.ActivationFunctionType.Relu
            )
            a1 = scratch.tile([1, NPC], F32, tag="a1")
            nc.vector.scalar_tensor_tensor(              # 1+|t|    (DVE)
                a1[:], at[:], 1.0, at[:],
                mybir.AluOpType.add, mybir.AluOpType.bypass,
            )
            vt = scratch.tile([1, NPC], F32, tag="vt")
            nc.vector.reciprocal_approx_fast(out=vt[:], in_=a1[:])
            sst = scratch.tile([1, NPC], F32, tag="sst")
            nc.vector.tensor_mul(sst[:], p[0:1, :], vt[:])  # softsign(t)
            nc.vector.copy_predicated(ot[:], m1_t[0:1, :], rt[:])
            nc.vector.copy_predicated(ot[:], m2_t[0:1, :], sst[:])

            nc.scalar.dma_start(o[:], ot[:])

    nc.compile()
    return nc


def kernel(**inputs) -> np.ndarray:
    global _BUILT, LAST_RESULTS

    iv = np.asarray(inputs["input_values"], dtype=np.float32)
    W = np.asarray(inputs["weight_matrix"], dtype=np.float32)
    bias = np.asarray(inputs["biases"], dtype=np.float32)
    act = np.asarray(inputs["act_ids"])
    iidx = np.asarray(inputs["input_indices"]).astype(np.int64)
    oidx = np.asarray(inputs["output_indices"]).astype(np.int64)

    n = W.shape[0]
    # Dense neuron-state vector (duplicate indices: last write wins, matching
    # jax's .at[].set) and its index support.
    states = np.zeros(n, np.float32)
    states[iidx] = iv
    live = np.zeros(n, dtype=bool)
    live[iidx] = True
    support = np.flatnonzero(live)
    assert support.size <= K, "more than K live rows not supported"
    rows = np.zeros(K, np.int64)          # pad with row 0 (x=0 there => no-op)
    rows[: support.size] = support
    xvec = np.zeros(K, np.float32)
    xvec[: support.size] = states[support]

    xh_t = np.ascontiguousarray(
        xvec.astype(np.float16).reshape(KC, 128).T
    )                                                     # [128, KC]

    in_maps = []
    for c in range(N_CORES):
        cols = oidx[c * NPC : (c + 1) * NPC]
        ws = W[np.ix_(rows, cols)].astype(np.float16)     # [K, NPC]
        # [K, NPC] -> [p, kc, col]: partition p holds row kc*128+p
        wh = np.ascontiguousarray(
            ws.reshape(KC, 128, NPC).transpose(1, 0, 2)
        ).reshape(128, KC * NPC)
        in_maps.append(
            {
                "w": wh,
                "xh": xh_t,
                "b": bias[cols].reshape(1, NPC).astype(np.float32),
                "m1": (act[cols] == 1).astype(np.uint8).reshape(1, NPC),
                "m2": (act[cols] == 2).astype(np.uint8).reshape(1, NPC),
            }
        )

    if _BUILT is None:
        _BUILT = _build_bass()
    LAST_RESULTS = run_bass_kernel_spmd(
        _BUILT, in_maps, core_ids=list(range(N_CORES))
    )
    return np.concatenate(
        [LAST_RESULTS.results[c]["o"][0] for c in range(N_CORES)]
    ).astype(np.float32)


# revision 44
# speedup vs baseline: 1.0230x; 1.0001x over previous
"""Trainium2 Bass kernel for the dense GNN message-passing step.

Computation (N=16384, NUM_IN=1024, NUM_OUT=256):
    states = zeros(N); states[input_indices] = input_values
    total  = states @ W + biases                      # GEMV over [N, N] f32
    out    = act_select(total)[output_indices]        # 0=id, 1=relu, 2=softsign

Strategy:
  * `states` is zero outside the (<=1024) positions named by input_indices,
    so only those rows of W contribute to the GEMV (host packs the live
    rows, padded to K=1024).
  * Only output_indices (256 of 16384) of the result are ever read, so
    only those COLUMNS of W are needed: the host gathers
    W[rows][:, output_indices] -> [1024, 256] (0.5 MB as fp16 instead of
    1 GB), and the device GEMV + bias + activation runs on just those
    neurons. The gathered column order IS the output order, so the
    host-side epilogue is a plain concat of the per-core slices.
  * The 256 columns are sharded 32-per-core across the 8 cores (tensor
    parallel per the sharding hint, applied post-gather).
  * Single-pass fp16 GEMV: x and W rounded to fp16, products accumulate
    exactly in f32 PSUM. Measured end-to-end error ~1.3e-4 relative
    (vs the 2e-2 gate); bias is added exactly via an f32 ones-row
    matmul into the same accumulation group.
  * Instruction count is kept minimal: the profile shows a fixed
    end-of-kernel drain that costs ~115 ns per queue instruction, so
    fewer instructions shrink the measured window directly.
  * Epilogue on [1,32] straight out of PSUM: |t| -> 1+|t| ->
    reciprocal_approx_fast (~51 ULP, plenty here) -> t*recip, with
    relu/softsign/identity selected by host-precomputed uint8 masks.
"""

import sys
import types

import numpy as np
from contextlib import ExitStack

import concourse.bacc as bacc
import concourse.tile as tile
from concourse import mybir
from concourse.bass_utils import run_bass_kernel_spmd


def _ensure_ntff_hook_module():
    """bass_utils imports antenv.axon_hooks when BASS_TRACE=1; some agent
    images ship antenv without that submodule, which would crash the run
    instead of degrading to trace-skip. Install a shim (backed by
    trn_agent_boot's ctypes NTFF driver when present) only if the real
    module is missing."""
    try:
        import antenv.axon_hooks  # noqa: F401
        return
    except ImportError:
        pass
    hook = [None]
    mod = types.ModuleType("antenv.axon_hooks")
    mod.set_axon_ntff_profile_hook = lambda h: hook.__setitem__(0, h)
    mod.get_axon_ntff_profile_hook = lambda: hook[0]
    try:
        import antenv as _antenv
        from trn_agent_boot.trn_boot import _ntff_profile_via_ctypes

        mod.set_axon_ntff_profile_hook(
            _ntff_profile_via_ctypes("/opt/axon/libaxon_pjrt.so")
        )
        sys.modules["antenv.axon_hooks"] = mod
        _antenv.axon_hooks = mod
    except Exception:
        sys.modules.setdefault("antenv.axon_hooks", mod)


_ensure_ntff_hook_module()

N_CORES = 8
K = 1024                 # padded contraction size (live rows)
KC = K // 128            # 8 k-chunks
NOUT = 256               # gathered output neurons
NPC = NOUT // N_CORES    # 32 output columns per core
F32 = mybir.dt.float32
F16 = mybir.dt.float16
U8 = mybir.dt.uint8

_BUILT = None            # cached nc so repeat calls reuse the compiled module
LAST_RESULTS = None      # BassKernelResults of the most recent run (for test.py)


def _build_bass():
    nc = bacc.Bacc(
        "TRN2", target_bir_lowering=False, debug=False, num_devices=N_CORES
    )
    # w layout: [p, kc*col] — 512 B contiguous per partition.
    w = nc.dram_tensor("w", [128, KC * NPC], F16, kind="ExternalInput").ap()
    xh = nc.dram_tensor("xh", [128, KC], F16, kind="ExternalInput").ap()
    b = nc.dram_tensor("b", [1, NPC], F32, kind="ExternalInput").ap()
    m1 = nc.dram_tensor("m1", [1, NPC], U8, kind="ExternalInput").ap()
    m2 = nc.dram_tensor("m2", [1, NPC], U8, kind="ExternalInput").ap()
    o = nc.dram_tensor("o", [1, NPC], F32, kind="ExternalOutput").ap()

    with tile.TileContext(nc) as tc:
        with ExitStack() as ctx:
            small = ctx.enter_context(tc.tile_pool(name="small", bufs=1))
            wpool = ctx.enter_context(tc.tile_pool(name="wp", bufs=1))
            ppool = ctx.enter_context(tc.tile_pool(name="pp", bufs=1, space="PSUM"))
            scratch = ctx.enter_context(tc.tile_pool(name="scr", bufs=1))

            # Spread input DMAs across queues so their descriptor
            # generations (~0.7 us each of sequencer time) run in
            # parallel; W split in two keeps the DMA engines streaming
            # densely (single big DMAs were observed to trickle).
            xh_t = small.tile([128, KC], F16, tag="xh")
            nc.scalar.dma_start(xh_t[:], xh[:])
            w_t = wpool.tile([128, KC * NPC], F16, tag="w")
            half = (KC // 2) * NPC
            nc.sync.dma_start(w_t[:, 0:half], w[:, 0:half])
            nc.sync.dma_start(w_t[:, half:], w[:, half:])
            b_t = small.tile([1, NPC], F32, tag="bt")
            nc.gpsimd.dma_start(b_t[:], b[:])
            m1_t = small.tile([1, NPC], U8, tag="m1t")
            nc.gpsimd.dma_start(m1_t[:], m1[:])
            m2_t = small.tile([1, NPC], U8, tag="m2t")
            nc.gpsimd.dma_start(m2_t[:], m2[:])
            ones_t = small.tile([1, 1], F32, tag="ones")
            nc.gpsimd.memset(ones_t[:], 1.0)

            p = ppool.tile([1, NPC], F32, tag="p")

            # p = b + x'W  (bias exact via f32 ones-row matmul)
            nc.tensor.matmul(
                p[0:1, :], ones_t[0:1, :], b_t[0:1, :],
                start=True, stop=False,
            )
            for kc in range(KC):
                nc.tensor.matmul(
                    p[0:1, :], xh_t[:, kc : kc + 1],
                    w_t[:, kc * NPC : (kc + 1) * NPC],
                    start=False, stop=(kc == KC - 1),
                )

            # t = p; softsign chain on DVE, identity/relu copies on ACT.
            at = scratch.tile([1, NPC], F32, tag="at")
            nc.scalar.activation(                        # |t|      (ACT)
                at[:], p[0:1, :], mybir.ActivationFunctionType.Abs
            )
            ot = scratch.tile([1, NPC], F32, tag="ot")
            nc.scalar.copy(ot[:], p[0:1, :])             # t        (ACT)
            rt = scratch.tile([1, NPC], F32, tag="rt")
            nc.scalar.activation(                        # relu(t)  (ACT)
                rt[:], p[0:1, :], mybir.ActivationFunctionType.Relu
            )
            a1 = scratch.tile([1, NPC], F32, tag="a1")
            nc.vector.scalar_tensor_tensor(              # 1+|t|    (DVE)
                a1[:], at[:], 1.0, at[:],
                mybir.AluOpType.add, mybir.AluOpType.bypass,
            )
            vt = scratch.tile([1, NPC], F32, tag="vt")
            nc.vector.reciprocal_approx_fast(out=vt[:], in_=a1[:])
            sst = scratch.tile([1, NPC], F32, tag="sst")
            nc.vector.tensor_mul(sst[:], p[0:1, :], vt[:])  # softsign(t)
            nc.vector.copy_predicated(ot[:], m1_t[0:1, :], rt[:])
            nc.vector.copy_predicated(ot[:], m2_t[0:1, :], sst[:])

            nc.scalar.dma_start(o[:], ot[:])

    nc.compile()
    return nc


def kernel(**inputs) -> np.ndarray:
    global _BUILT, LAST_RESULTS

    iv = np.asarray(inputs["input_values"], dtype=np.float32)
    W = np.asarray(inputs["weight_matrix"], dtype=np.float32)
    bias = np.asarray(inputs["biases"], dtype=np.float32)
    act = np.asarray(inputs["act_ids"])
    iidx = np.asarray(inputs["input_indices"]).astype(np.int64)
    oidx = np.asarray(inputs["output_indices"]).astype(np.int64)

    n = W.shape[0]
    # Dense neuron-state vector (duplicate indices: last write wins, matching
    # jax's .at[].set) and its index support.
    states = np.zeros(n, np.float32)
    states[iidx] = iv
    live = np.zeros(n, dtype=bool)
    live[iidx] = True
    support = np.flatnonzero(live)
    assert support.size <= K, "more than K live rows not supported"
    rows = np.zeros(K, np.int64)          # pad with row 0 (x=0 there => no-op)
    rows[: support.size] = support
    xvec = np.zeros(K, np.float32)
    xvec[: support.size] = states[support]

    xh_t = np.ascontiguousarray(
        xvec.astype(np.float16).reshape(KC, 128).T
    )                                                     # [128, KC]

    in_maps = []
    for c in range(N_CORES):
        cols = oidx[c * NPC : (c + 1) * NPC]
        ws = W[np.ix_(rows, cols)].astype(np.float16)     # [K, NPC]
        # [K, NPC] -> [p, kc, col]: partition p holds row kc*128+p
        wh = np.ascontiguousarray(
            ws.reshape(KC, 128, NPC).transpose(1, 0, 2)
        ).reshape(128, KC * NPC)
        in_maps.append(
            {
                "w": wh,
                "xh": xh_t,
                "b": bias[cols].reshape(1, NPC).astype(np.float32),
                "m1": (act[cols] == 1).astype(np.uint8).reshape(1, NPC),
                "m2": (act[cols] == 2).astype(np.uint8).reshape(1, NPC),
            }
        )

    if _BUILT is None:
        _BUILT = _build_bass()
    LAST_RESULTS = run_bass_kernel_spmd(
        _BUILT, in_maps, core_ids=list(range(N_CORES))
    )
    return np.concatenate(
        [LAST_RESULTS.results[c]["o"][0] for c in range(N_CORES)]
    ).astype(np.float32)


# revision 47
# speedup vs baseline: 1.0237x; 1.0007x over previous
"""Trainium2 Bass kernel for the dense GNN message-passing step.

Computation (N=16384, NUM_IN=1024, NUM_OUT=256):
    states = zeros(N); states[input_indices] = input_values
    total  = states @ W + biases                      # GEMV over [N, N] f32
    out    = act_select(total)[output_indices]        # 0=id, 1=relu, 2=softsign

Strategy:
  * `states` is zero outside the (<=1024) positions named by input_indices,
    so only those rows of W contribute to the GEMV (host packs the live
    rows, padded to K=1024).
  * Only output_indices (256 of 16384) of the result are ever read, so
    only those COLUMNS of W are needed: the host gathers
    W[rows][:, output_indices] -> [1024, 256] (0.5 MB as fp16 instead of
    1 GB), and the device GEMV + bias + activation runs on just those
    neurons. The gathered column order IS the output order, so the
    host-side epilogue is a plain concat of the per-core slices.
  * The 256 columns are sharded 32-per-core across the 8 cores (tensor
    parallel per the sharding hint, applied post-gather).
  * Single-pass fp16 GEMV: x and W rounded to fp16, products accumulate
    exactly in f32 PSUM. Measured end-to-end error ~1.3e-4 relative
    (vs the 2e-2 gate); bias is added exactly via an f32 ones-row
    matmul into the same accumulation group.
  * Instruction count is kept minimal: the profile shows a fixed
    end-of-kernel drain that costs ~115 ns per queue instruction, so
    fewer instructions shrink the measured window directly.
  * Epilogue on [1,32] straight out of PSUM: |t| -> 1+|t| ->
    reciprocal_approx_fast (~51 ULP, plenty here) -> t*recip, with
    relu/softsign/identity selected by host-precomputed uint8 masks.
"""

import sys
import types

import numpy as np
from contextlib import ExitStack

import concourse.bacc as bacc
import concourse.tile as tile
from concourse import mybir
from concourse.bass_utils import run_bass_kernel_spmd


def _ensure_ntff_hook_module():
    """bass_utils imports antenv.axon_hooks when BASS_TRACE=1; some agent
    images ship antenv without that submodule, which would crash the run
    instead of degrading to trace-skip. Install a shim (backed by
    trn_agent_boot's ctypes NTFF driver when present) only if the real
    module is missing."""
    try:
        import antenv.axon_hooks  # noqa: F401
        return
    except ImportError:
        pass
    hook = [None]
    mod = types.ModuleType("antenv.axon_hooks")
    mod.set_axon_ntff_profile_hook = lambda h: hook.__setitem__(0, h)
    mod.get_axon_ntff_profile_hook = lambda: hook[0]
    try:
        import antenv as _antenv
        from trn_agent_boot.trn_boot import _ntff_profile_via_ctypes

        mod.set_axon_ntff_profile_hook(
            _ntff_profile_via_ctypes("/opt/axon/libaxon_pjrt.so")
        )
        sys.modules["antenv.axon_hooks"] = mod
        _antenv.axon_hooks = mod
    except Exception:
        sys.modules.setdefault("antenv.axon_hooks", mod)


_ensure_ntff_hook_module()

N_CORES = 8
K = 1024                 # padded contraction size (live rows)
KC = K // 128            # 8 k-chunks
NOUT = 256               # gathered output neurons
NPC = NOUT // N_CORES    # 32 output columns per core
F32 = mybir.dt.float32
F16 = mybir.dt.float16
U8 = mybir.dt.uint8

_BUILT = None            # cached nc so repeat calls reuse the compiled module
LAST_RESULTS = None      # BassKernelResults of the most recent run (for test.py)


def _build_bass():
    nc = bacc.Bacc(
        "TRN2", target_bir_lowering=False, debug=False, num_devices=N_CORES
    )
    # w layout: [p, kc*col] — 512 B contiguous per partition.
    w = nc.dram_tensor("w", [128, KC * NPC], F16, kind="ExternalInput").ap()
    xh = nc.dram_tensor("xh", [128, KC], F16, kind="ExternalInput").ap()
    b = nc.dram_tensor("b", [1, NPC], F32, kind="ExternalInput").ap()
    m1 = nc.dram_tensor("m1", [1, NPC], U8, kind="ExternalInput").ap()
    m2 = nc.dram_tensor("m2", [1, NPC], U8, kind="ExternalInput").ap()
    o = nc.dram_tensor("o", [1, NPC], F32, kind="ExternalOutput").ap()

    with tile.TileContext(nc) as tc:
        with ExitStack() as ctx:
            small = ctx.enter_context(tc.tile_pool(name="small", bufs=1))
            wpool = ctx.enter_context(tc.tile_pool(name="wp", bufs=1))
            ppool = ctx.enter_context(tc.tile_pool(name="pp", bufs=1, space="PSUM"))
            scratch = ctx.enter_context(tc.tile_pool(name="scr", bufs=1))

            # Spread input DMAs across queues so their descriptor
            # generations (~0.7 us each of sequencer time) run in
            # parallel; W split in two keeps the DMA engines streaming
            # densely (single big DMAs were observed to trickle).
            xh_t = small.tile([128, KC], F16, tag="xh")
            nc.scalar.dma_start(xh_t[:], xh[:])
            w_t = wpool.tile([128, KC * NPC], F16, tag="w")
            half = (KC // 2) * NPC
            nc.sync.dma_start(w_t[:, 0:half], w[:, 0:half])
            nc.sync.dma_start(w_t[:, half:], w[:, half:])
            b_t = small.tile([1, NPC], F32, tag="bt")
            nc.gpsimd.dma_start(b_t[:], b[:])
            m1_t = small.tile([1, NPC], U8, tag="m1t")
            nc.gpsimd.dma_start(m1_t[:], m1[:])
            m2_t = small.tile([1, NPC], U8, tag="m2t")
            nc.gpsimd.dma_start(m2_t[:], m2[:])
            ones_t = small.tile([1, 1], F32, tag="ones")
            nc.gpsimd.memset(ones_t[:], 1.0)

            p = ppool.tile([1, NPC], F32, tag="p")

            # p = b + x'W  (bias exact via f32 ones-row matmul)
            nc.tensor.matmul(
                p[0:1, :], ones_t[0:1, :], b_t[0:1, :],
                start=True, stop=False,
            )
            for kc in range(KC):
                nc.tensor.matmul(
                    p[0:1, :], xh_t[:, kc : kc + 1],
                    w_t[:, kc * NPC : (kc + 1) * NPC],
                    start=False, stop=(kc == KC - 1),
                )

            # t = p; softsign chain on DVE, identity/relu copies on ACT.
            at = scratch.tile([1, NPC], F32, tag="at")
            nc.scalar.activation(                        # |t|      (ACT)
                at[:], p[0:1, :], mybir.ActivationFunctionType.Abs
            )
            ot = scratch.tile([1, NPC], F32, tag="ot")
            nc.scalar.copy(ot[:], p[0:1, :])             # t        (ACT)
            rt = scratch.tile([1, NPC], F32, tag="rt")
            nc.scalar.activation(                        # relu(t)  (ACT)
                rt[:], p[0:1, :], mybir.ActivationFunctionType.Relu
            )
            a1 = scratch.tile([1, NPC], F32, tag="a1")
            nc.vector.scalar_tensor_tensor(              # 1+|t|    (DVE)
                a1[:], at[:], 1.0, at[:],
                mybir.AluOpType.add, mybir.AluOpType.bypass,
            )
            vt = scratch.tile([1, NPC], F32, tag="vt")
            nc.vector.reciprocal_approx_fast(out=vt[:], in_=a1[:])
            sst = scratch.tile([1, NPC], F32, tag="sst")
            nc.vector.tensor_mul(sst[:], p[0:1, :], vt[:])  # softsign(t)
            nc.vector.copy_predicated(ot[:], m1_t[0:1, :], rt[:])
            nc.vector.copy_predicated(ot[:], m2_t[0:1, :], sst[:])

            nc.scalar.dma_start(o[:], ot[:])

    nc.compile()
    return nc


def kernel(**inputs) -> np.ndarray:
    global _BUILT, LAST_RESULTS

    iv = np.asarray(inputs["input_values"], dtype=np.float32)
    W = np.asarray(inputs["weight_matrix"], dtype=np.float32)
    bias = np.asarray(inputs["biases"], dtype=np.float32)
    act = np.asarray(inputs["act_ids"])
    iidx = np.asarray(inputs["input_indices"]).astype(np.int64)
    oidx = np.asarray(inputs["output_indices"]).astype(np.int64)

    n = W.shape[0]
    # Dense neuron-state vector (duplicate indices: last write wins, matching
    # jax's .at[].set) and its index support.
    states = np.zeros(n, np.float32)
    states[iidx] = iv
    live = np.zeros(n, dtype=bool)
    live[iidx] = True
    support = np.flatnonzero(live)
    assert support.size <= K, "more than K live rows not supported"
    rows = np.zeros(K, np.int64)          # pad with row 0 (x=0 there => no-op)
    rows[: support.size] = support
    xvec = np.zeros(K, np.float32)
    xvec[: support.size] = states[support]

    xh_t = np.ascontiguousarray(
        xvec.astype(np.float16).reshape(KC, 128).T
    )                                                     # [128, KC]

    in_maps = []
    for c in range(N_CORES):
        cols = oidx[c * NPC : (c + 1) * NPC]
        ws = W[np.ix_(rows, cols)].astype(np.float16)     # [K, NPC]
        # [K, NPC] -> [p, kc, col]: partition p holds row kc*128+p
        wh = np.ascontiguousarray(
            ws.reshape(KC, 128, NPC).transpose(1, 0, 2)
        ).reshape(128, KC * NPC)
        in_maps.append(
            {
                "w": wh,
                "xh": xh_t,
                "b": bias[cols].reshape(1, NPC).astype(np.float32),
                "m1": (act[cols] == 1).astype(np.uint8).reshape(1, NPC),
                "m2": (act[cols] == 2).astype(np.uint8).reshape(1, NPC),
            }
        )

    if _BUILT is None:
        _BUILT = _build_bass()
    LAST_RESULTS = run_bass_kernel_spmd(
        _BUILT, in_maps, core_ids=list(range(N_CORES))
    )
    return np.concatenate(
        [LAST_RESULTS.results[c]["o"][0] for c in range(N_CORES)]
    ).astype(np.float32)


# revision 48
# speedup vs baseline: 1.0621x; 1.0374x over previous
"""Trainium2 Bass kernel for the dense GNN message-passing step.

Computation (N=16384, NUM_IN=1024, NUM_OUT=256):
    states = zeros(N); states[input_indices] = input_values
    total  = states @ W + biases                      # GEMV over [N, N] f32
    out    = act_select(total)[output_indices]        # 0=id, 1=relu, 2=softsign

Strategy:
  * `states` is zero outside the (<=1024) positions named by input_indices,
    so only those rows of W contribute to the GEMV (host packs the live
    rows, padded to K=1024).
  * Only output_indices (256 of 16384) of the result are ever read, so
    only those COLUMNS of W are needed: the host gathers
    W[rows][:, output_indices] -> [1024, 256] (0.5 MB as fp16 instead of
    1 GB), and the device GEMV + bias + activation runs on just those
    neurons. The gathered column order IS the output order, so the
    host-side epilogue is a plain concat of the per-core slices.
  * The 256 columns are sharded 32-per-core across the 8 cores (tensor
    parallel per the sharding hint, applied post-gather).
  * Single-pass fp16 GEMV: x and W rounded to fp16, products accumulate
    exactly in f32 PSUM. Measured end-to-end error ~1.3e-4 relative
    (vs the 2e-2 gate); bias is added exactly via an f32 ones-row
    matmul into the same accumulation group.
  * Instruction count is kept minimal: the profile shows a fixed
    end-of-kernel drain that costs ~115 ns per queue instruction, so
    fewer instructions shrink the measured window directly.
  * Epilogue on [1,32] straight out of PSUM: |t| -> 1+|t| ->
    reciprocal_approx_fast (~51 ULP, plenty here) -> t*recip, with
    relu/softsign/identity selected by host-precomputed uint8 masks.
"""

import sys
import types

import numpy as np
from contextlib import ExitStack

import concourse.bacc as bacc
import concourse.tile as tile
from concourse import mybir
from concourse.bass_utils import run_bass_kernel_spmd


def _ensure_ntff_hook_module():
    """bass_utils imports antenv.axon_hooks when BASS_TRACE=1; some agent
    images ship antenv without that submodule, which would crash the run
    instead of degrading to trace-skip. Install a shim (backed by
    trn_agent_boot's ctypes NTFF driver when present) only if the real
    module is missing."""
    try:
        import antenv.axon_hooks  # noqa: F401
        return
    except ImportError:
        pass
    hook = [None]
    mod = types.ModuleType("antenv.axon_hooks")
    mod.set_axon_ntff_profile_hook = lambda h: hook.__setitem__(0, h)
    mod.get_axon_ntff_profile_hook = lambda: hook[0]
    try:
        import antenv as _antenv
        from trn_agent_boot.trn_boot import _ntff_profile_via_ctypes

        mod.set_axon_ntff_profile_hook(
            _ntff_profile_via_ctypes("/opt/axon/libaxon_pjrt.so")
        )
        sys.modules["antenv.axon_hooks"] = mod
        _antenv.axon_hooks = mod
    except Exception:
        sys.modules.setdefault("antenv.axon_hooks", mod)


_ensure_ntff_hook_module()

N_CORES = 8
K = 1024                 # padded contraction size (live rows)
KC = K // 128            # 8 k-chunks
NOUT = 256               # gathered output neurons
NPC = NOUT // N_CORES    # 32 output columns per core
F32 = mybir.dt.float32
F16 = mybir.dt.float16
U8 = mybir.dt.uint8

_BUILT = None            # cached nc so repeat calls reuse the compiled module
LAST_RESULTS = None      # BassKernelResults of the most recent run (for test.py)


def _build_bass():
    nc = bacc.Bacc(
        "TRN2", target_bir_lowering=False, debug=False, num_devices=N_CORES
    )
    # w layout: [p, x(8) | kc*col] — x rides as the first 16 bytes of each
    # partition's W stripe, so it streams with the dense W DMA instead of a
    # separate 128x16B-packet transfer that landed late and jittered.
    w = nc.dram_tensor("w", [128, KC + KC * NPC], F16, kind="ExternalInput").ap()
    b = nc.dram_tensor("b", [1, NPC], F32, kind="ExternalInput").ap()
    m1 = nc.dram_tensor("m1", [1, NPC], U8, kind="ExternalInput").ap()
    m2 = nc.dram_tensor("m2", [1, NPC], U8, kind="ExternalInput").ap()
    o = nc.dram_tensor("o", [1, NPC], F32, kind="ExternalOutput").ap()

    with tile.TileContext(nc) as tc:
        with ExitStack() as ctx:
            small = ctx.enter_context(tc.tile_pool(name="small", bufs=1))
            wpool = ctx.enter_context(tc.tile_pool(name="wp", bufs=1))
            ppool = ctx.enter_context(tc.tile_pool(name="pp", bufs=1, space="PSUM"))
            scratch = ctx.enter_context(tc.tile_pool(name="scr", bufs=1))

            # Spread input DMAs across queues so their descriptor
            # generations (~0.7 us each of sequencer time) run in
            # parallel; W split in two keeps the DMA engines streaming
            # densely (single big DMAs were observed to trickle).
            w_t = wpool.tile([128, KC + KC * NPC], F16, tag="w")
            half = KC + (KC // 2) * NPC
            nc.sync.dma_start(w_t[:, 0:half], w[:, 0:half])
            nc.sync.dma_start(w_t[:, half:], w[:, half:])
            xh_t = w_t[:, 0:KC]
            b_t = small.tile([1, NPC], F32, tag="bt")
            nc.gpsimd.dma_start(b_t[:], b[:])
            m1_t = small.tile([1, NPC], U8, tag="m1t")
            nc.gpsimd.dma_start(m1_t[:], m1[:])
            m2_t = small.tile([1, NPC], U8, tag="m2t")
            nc.gpsimd.dma_start(m2_t[:], m2[:])
            ones_t = small.tile([1, 1], F32, tag="ones")
            nc.gpsimd.memset(ones_t[:], 1.0)

            p = ppool.tile([1, NPC], F32, tag="p")

            # p = b + x'W  (bias exact via f32 ones-row matmul)
            nc.tensor.matmul(
                p[0:1, :], ones_t[0:1, :], b_t[0:1, :],
                start=True, stop=False,
            )
            for kc in range(KC):
                nc.tensor.matmul(
                    p[0:1, :], xh_t[:, kc : kc + 1],
                    w_t[:, KC + kc * NPC : KC + (kc + 1) * NPC],
                    start=False, stop=(kc == KC - 1),
                )

            # t = p; softsign chain on DVE, identity/relu copies on ACT.
            at = scratch.tile([1, NPC], F32, tag="at")
            nc.scalar.activation(                        # |t|      (ACT)
                at[:], p[0:1, :], mybir.ActivationFunctionType.Abs
            )
            ot = scratch.tile([1, NPC], F32, tag="ot")
            nc.scalar.copy(ot[:], p[0:1, :])             # t        (ACT)
            rt = scratch.tile([1, NPC], F32, tag="rt")
            nc.scalar.activation(                        # relu(t)  (ACT)
                rt[:], p[0:1, :], mybir.ActivationFunctionType.Relu
            )
            a1 = scratch.tile([1, NPC], F32, tag="a1")
            nc.vector.scalar_tensor_tensor(              # 1+|t|    (DVE)
                a1[:], at[:], 1.0, at[:],
                mybir.AluOpType.add, mybir.AluOpType.bypass,
            )
            vt = scratch.tile([1, NPC], F32, tag="vt")
            nc.vector.reciprocal_approx_fast(out=vt[:], in_=a1[:])
            sst = scratch.tile([1, NPC], F32, tag="sst")
            nc.vector.tensor_mul(sst[:], p[0:1, :], vt[:])  # softsign(t)
            nc.vector.copy_predicated(ot[:], m1_t[0:1, :], rt[:])
            nc.vector.copy_predicated(ot[:], m2_t[0:1, :], sst[:])

            nc.scalar.dma_start(o[:], ot[:])

    nc.compile()
    return nc


def kernel(**inputs) -> np.ndarray:
    global _BUILT, LAST_RESULTS

    iv = np.asarray(inputs["input_values"], dtype=np.float32)
    W = np.asarray(inputs["weight_matrix"], dtype=np.float32)
    bias = np.asarray(inputs["biases"], dtype=np.float32)
    act = np.asarray(inputs["act_ids"])
    iidx = np.asarray(inputs["input_indices"]).astype(np.int64)
    oidx = np.asarray(inputs["output_indices"]).astype(np.int64)

    n = W.shape[0]
    # Dense neuron-state vector (duplicate indices: last write wins, matching
    # jax's .at[].set) and its index support.
    states = np.zeros(n, np.float32)
    states[iidx] = iv
    live = np.zeros(n, dtype=bool)
    live[iidx] = True
    support = np.flatnonzero(live)
    assert support.size <= K, "more than K live rows not supported"
    rows = np.zeros(K, np.int64)          # pad with row 0 (x=0 there => no-op)
    rows[: support.size] = support
    xvec = np.zeros(K, np.float32)
    xvec[: support.size] = states[support]

    xh_t = np.ascontiguousarray(
        xvec.astype(np.float16).reshape(KC, 128).T
    )                                                     # [128, KC]

    in_maps = []
    for c in range(N_CORES):
        cols = oidx[c * NPC : (c + 1) * NPC]
        ws = W[np.ix_(rows, cols)].astype(np.float16)     # [K, NPC]
        # [K, NPC] -> [p, kc, col]: partition p holds row kc*128+p
        wh = np.ascontiguousarray(
            ws.reshape(KC, 128, NPC).transpose(1, 0, 2)
        ).reshape(128, KC * NPC)
        in_maps.append(
            {
                "w": np.ascontiguousarray(np.concatenate([xh_t, wh], axis=1)),
                "b": bias[cols].reshape(1, NPC).astype(np.float32),
                "m1": (act[cols] == 1).astype(np.uint8).reshape(1, NPC),
                "m2": (act[cols] == 2).astype(np.uint8).reshape(1, NPC),
            }
        )

    if _BUILT is None:
        _BUILT = _build_bass()
    LAST_RESULTS = run_bass_kernel_spmd(
        _BUILT, in_maps, core_ids=list(range(N_CORES))
    )
    return np.concatenate(
        [LAST_RESULTS.results[c]["o"][0] for c in range(N_CORES)]
    ).astype(np.float32)
